# revision 30
# baseline (speedup 1.0000x reference)
"""Trainium2 Bass kernel for nn_BCIM_45861660787130 (pooling / box-filter sim).

Math per sample (C=128 channels, 32x32 spatial = S=1024 pixels):
  unit = p / ||p||_C
  wmean = 3x3 zero-padded box mean of unit (per channel)
  sim = <unit, wmean>_C          # per pixel
  out = p * sim, then channel deinterleave c=(f*2+e) -> [e*S + s, f]

Design (per core, data-parallel over batch; 8 samples per group, NS=8):
  - SWDGE cast-DMA loads each group as bf16 [c=128, (b, s)] (two s-half
    DMAs so the pipeline ramps at 1 MB).
  - Flat software pipeline over all 32 chunk-iterations (4 groups x 8
    s-chunks) with explicit stage lags (transpose @t, square/ss @t-1,
    sqrt/rinv @t-2, normalize @t-3, box/dot @t-5, scale-out @t-6) so
    every cross-engine dependency is >=1 iteration old and the in-order
    engine queues never head-of-line block.
  - PE: bf16 transposes (1 cyc/row) -> pT [s,c] PSUM; box filter as
    block-tridiagonal bf16 matmuls box_k = Bd^T u_k + Bp^T u_{k-1} +
    Bn^T u_{k+1} (PSUM accumulate, N=512 halves per bank).
  - ACT: Square [128,1024] -> sq; sqrt -> nrm; 1 normalize copy-scale;
    7 of 8 scale-out copies.  DVE: segmented reduces (ss, z), recip,
    u-normalize as one stride-0-broadcast TT (write AP deinterleaves
    c=(f*2+e) -> (e*64+f) so downstream is contiguous), wscr = u*box.
    Pool: 1 of 8 scale-out slots (Pool ops cost ~2.9us fixed; only one
    slot is profitable).
  - NOTE hard-won HW constraints: tensor_tensor_reduce crashes the
    device; GPSIMD cannot touch PSUM; one matmul's PSUM out must fit a
    2 KB bank.
  - Output staged bf16 (host upcasts); per (sample, e, k-half) HWDGE
    DMAs so the tail drains early.  rel err ~3.6e-3 (bf16 input quant).
"""

import os
import sys

sys.path.insert(0, "/opt/trn_rl_repo")

import numpy as np

from concourse import bacc, bass, mybir, tile
from concourse.bass import broadcast_tensor_aps
from concourse.bass_utils import run_bass_kernel_spmd

F32 = mybir.dt.float32
BF16 = mybir.dt.bfloat16
AF = mybir.ActivationFunctionType
ALU = mybir.AluOpType
AX = mybir.AxisListType

B_PER_CORE = 32  # samples per core
NS = int(os.environ.get("NS", 8))  # samples per group
NG = B_PER_CORE // NS
NG_RUN = int(os.environ.get("NG_RUN", NG))
NK = 8  # s-chunks per sample (1024 / 128)
C = 128
S = 1024

# engine assignment knobs (A=ACT, D=DVE, P=Pool) per sample index
OUT_ASSIGN = (os.environ.get("OUT_ASSIGN", "A" * 16) * 4)[:16]
SS_ENG = os.environ.get("SS_ENG", "D")  # segmented sum-sq reduce: D or P
TTR_ASSIGN = (os.environ.get("TTR_ASSIGN", "D" * 16) * 4)[:16]
IN_BF16 = os.environ.get("IN_BF16", "1") == "1"  # SWDGE cast input + bf16 transposes
U_MODE = os.environ.get("U_MODE", "bcast")  # bcast: one TT w/ stride-0; ts: per-sample
DEINT = os.environ.get("DEINT", "1") == "1"  # deinterleave at u write vs at out read
Z_MODE = os.environ.get("Z_MODE", "split")  # ttr: per-sample TTR (HW-crashes); split: TT+reduce
U_BF16 = os.environ.get("U_BF16", "1") == "1"  # u/wbox dtype bf16 vs f32r
W_ENG = os.environ.get("W_ENG", "D")  # wscr big TT: D=DVE, P=Pool
OUT_BF16 = os.environ.get("OUT_BF16", "1") == "1"  # bf16 DRAM output, host upcast
UK = int(os.environ.get("UK", min(NS, 5)))  # u: first UK samples DVE-bcast, rest ACT
SQW_BF16 = os.environ.get("SQW_BF16", "0") == "1"  # bf16 sq/wscr reduce inputs
SS_PE = os.environ.get("SS_PE", "1") == "1"  # sum-of-squares via PE ones-matmul
OUT_POOL2 = int(os.environ.get("OUT_POOL2", "3"))  # first N out slots as one Pool bcast TT
U_ASSIGN = (os.environ.get("U_ASSIGN", "D" * 16) * 4)[:16]


def _consts():
    t32 = (np.abs(np.subtract.outer(np.arange(32), np.arange(32))) <= 1).astype(
        np.float32
    )
    a4 = (np.abs(np.subtract.outer(np.arange(4), np.arange(4))) <= 1).astype(
        np.float32
    )
    e30 = np.zeros((4, 4), np.float32)
    e30[3, 0] = 1.0
    e03 = np.zeros((4, 4), np.float32)
    e03[0, 3] = 1.0
    bd = np.kron(a4, t32) / 9.0
    bp = np.kron(e30, t32) / 9.0  # from chunk k-1
    bn = np.kron(e03, t32) / 9.0  # from chunk k+1
    ident = np.eye(128, dtype=np.float32)
    wbox = np.stack([bd, bp, bn]).astype(np.float32)
    return ident, wbox


def _bcast(src_ap, like_ap):
    """Broadcast src_ap (fewer/size-1 dims) against like_ap via stride-0."""
    a, b = broadcast_tensor_aps(like_ap, src_ap)
    return b


def build_nc():
    nc = bacc.Bacc()
    p_d = nc.declare_dram_parameter("p", [B_PER_CORE, C, S], F32, isOutput=False)
    ODT = BF16 if OUT_BF16 else F32
    out_d = nc.declare_dram_parameter(
        "out", [B_PER_CORE, 2, NK, 128, 64], ODT, isOutput=True
    )
    IDT0 = BF16 if IN_BF16 else F32
    ident_d = nc.declare_dram_parameter("ident", [128, 128], IDT0, isOutput=False)
    MMDT = BF16 if U_BF16 else mybir.dt.float32r
    wbox_d = nc.declare_dram_parameter("wbox", [3, 128, 128], MMDT, isOutput=False)

    with tile.TileContext(nc) as tc:
        with (
            tc.tile_pool(name="consts", bufs=1) as cpool,
            tc.tile_pool(name="pin", bufs=4) as pin,
            tc.tile_pool(name="upool", bufs=12) as upool,
            tc.tile_pool(name="sq", bufs=3) as sqpool,
            tc.tile_pool(name="wscr", bufs=4) as wpool,
            tc.tile_pool(name="outp", bufs=4) as outpool,
            tc.tile_pool(name="stats", bufs=6 * NK) as stats,
            tc.tile_pool(name="sqc", bufs=3) as sqcpool,
            tc.tile_pool(name="psT", bufs=3 if SS_PE else 4, space="PSUM") as psT,
            tc.tile_pool(name="psB", bufs=2, space="PSUM") as psB,
            tc.tile_pool(name="psS", bufs=1, space="PSUM") as psS,
        ):
            IDT = BF16 if IN_BF16 else F32
            ident = cpool.tile([128, 128], IDT, tag="ident")
            wbox = cpool.tile([128, 3, 128], MMDT, tag="wbox")
            nc.sync.dma_start(ident[:], ident_d[:])
            nc.sync.dma_start(wbox[:], wbox_d[:].transpose([1, 0, 2]))
            bd, bp, bn = wbox[:, 0, :], wbox[:, 1, :], wbox[:, 2, :]
            ones_c = cpool.tile([128, 1], BF16, tag="ones")
            nc.vector.memset(ones_c[:], 1.0)

            # startup observers: make PE's vector clock see both const-DMA
            # queue sems so steady-state matmuls never wait on them.
            scr1 = psB.tile([128, 1], F32, tag="box")
            nc.tensor.matmul(scr1[:], ident[:], ident[:, 0:1], start=True, stop=True)
            scr2 = psB.tile([128, 1], F32, tag="box")
            nc.tensor.matmul(
                scr2[:], wbox[:, 0, :], wbox[:, 0, 0:1], start=True, stop=True
            )

            PDT = BF16 if IN_BF16 else F32
            ptiles = []
            for g in range(NG_RUN):
                pg = pin.tile([C, NS, S], PDT, tag="pg", name=f"pg_{g}")
                src_ap = p_d[g * NS : (g + 1) * NS].transpose([1, 0, 2])
                H = S // 2
                for h in range(2):
                    sl = slice(h * H, (h + 1) * H)
                    if IN_BF16:
                        # SWDGE cast f32 -> bf16 during HBM->SBUF
                        nc.gpsimd.dma_start(pg[:, :, sl], src_ap[:, :, sl])
                    else:
                        nc.sync.dma_start(pg[:, :, sl], src_ap[:, :, sl])
                ptiles.append(pg)

            # flat software pipeline over all chunks t=(g,k); stage lags
            # keep every cross-engine dep >=1 iteration old so per-engine
            # in-order queues never head-of-line block.
            TOT = NG_RUN * NK
            L_SQ, L_SQRT, L_U, L_BOX, L_OUT = 1, 2, 3, 5, 7
            L_BW = 6
            boxes = {}
            pTs, sss, nrms, rinvs, us, zs, fss = {}, {}, {}, {}, {}, {}, {}
            sqcs = {}
            outts = {}
            SSR = 4
            ss_all = (
                psS.tile([128, SSR, NS], F32, tag="ssall", name="ss_all")
                if SS_PE
                else None
            )
            L_T = 2 if SS_PE else 0

            for t in range(TOT + L_OUT + 1):
                # --- stage OUT (oldest first for queue friendliness) ---
                tt = t - L_OUT
                if 0 <= tt < TOT:
                    g, k = divmod(tt, NK)
                    fs = fss.pop(tt)
                    u = us[tt]
                    outt = outts[g]
                    if OUT_POOL2 > 0:
                        nb = OUT_POOL2
                        ob = outt[:, 0:nb, k, :, :]
                        ub = u[:, 0:nb, :].rearrange("p b (e f) -> p b e f", e=2)
                        fb = (
                            fs[:, 0:nb]
                            .unsqueeze(2)
                            .unsqueeze(3)
                            .broadcast_to([128, nb, 2, 64])
                        )
                        nc.gpsimd.tensor_tensor(ob, ub, fb, op=ALU.mult)
                    for b in range(OUT_POOL2, NS):
                        ov = outt[:, b, k, :, :].rearrange("p e f -> p (e f)")
                        uv = u[:, b, :]
                        if not U_BF16:
                            uv = uv.bitcast(F32)
                        eng = OUT_ASSIGN[b]
                        if eng == "A":
                            nc.scalar.activation(
                                ov, uv, AF.Copy, scale=fs[:, b : b + 1]
                            )
                        elif eng == "P":
                            nc.gpsimd.tensor_scalar_mul(ov, uv, fs[:, b : b + 1])
                        else:
                            nc.vector.tensor_scalar_mul(ov, uv, fs[:, b : b + 1])
                    last_g = g == NG_RUN - 1
                    flush = []
                    if last_g and k == NK // 2 - 1:
                        flush = [slice(0, NK // 2)]
                    elif last_g and k == NK - 1:
                        flush = [slice(NK // 2, NK)]
                    elif not last_g and k == NK - 1:
                        flush = [slice(0, NK)]
                    for kl in flush:
                        for b in range(NS):
                            for e in range(2):
                                dst = out_d[g * NS + b, e, kl].transpose([1, 0, 2])
                                nc.sync.dma_start(dst, outt[:, b, kl, e, :])
                    if k == NK - 1:
                        for j in range(max(0, tt - NK), tt + 1):
                            us.pop(j, None)

                # --- stage WSCR + zred + fs (box is 1 iter old) ---
                tt = t - L_BW
                if 0 <= tt < TOT:
                    box = boxes.pop(tt)
                    z = stats.tile([128, NS], F32, tag="z")
                    wscr = wpool.tile(
                        [128, NS, 128], BF16 if SQW_BF16 else F32, tag="w"
                    )
                    uk = us[tt][:]
                    if not U_BF16:
                        uk = uk.bitcast(F32)
                    nc.vector.tensor_tensor(wscr[:], uk, box[:], op=ALU.mult)
                    nc.vector.tensor_reduce(z[:], wscr[:], axis=AX.X, op=ALU.add)
                    fs = stats.tile([128, NS], F32, tag="fs")
                    nc.vector.tensor_mul(fs[:], z[:], nrms[tt][:])
                    fss[tt] = fs

                # --- stage BOX matmuls ---
                tt = t - L_BOX
                if 0 <= tt < TOT:
                    g, k = divmod(tt, NK)
                    box = psB.tile([128, NS, 128], F32, tag="box")
                    mms = [(bd, tt)]
                    if k > 0:
                        mms.append((bp, tt - 1))
                    if k < NK - 1:
                        mms.append((bn, tt + 1))
                    for h in range(0, NS, 4):
                        sl = slice(h, min(h + 4, NS))
                        for i, (w, j) in enumerate(mms):
                            nc.tensor.matmul(
                                box[:, sl, :],
                                w,
                                us[j][:, sl, :],
                                start=(i == 0),
                                stop=(i == len(mms) - 1),
                            )
                    boxes[tt] = box

                # --- stage U (normalize) ---
                tt = t - L_U
                if 0 <= tt < TOT:
                    g, k = divmod(tt, NK)
                    pT = pTs.pop(tt)
                    rinv = rinvs.pop(tt)
                    u = upool.tile(
                        [128, NS, 128], MMDT, tag="u", name=f"u_{tt}"
                    )
                    in0 = pT[:].rearrange("p b (f e) -> p b e f", e=2)
                    outv = u[:].rearrange("p b (e f) -> p b e f", e=2)
                    if UK > 0:
                        rb = (
                            rinv[:, 0:UK]
                            .unsqueeze(2)
                            .unsqueeze(3)
                            .broadcast_to([128, UK, 2, 64])
                        )
                        nc.vector.tensor_tensor(
                            outv[:, 0:UK], in0[:, 0:UK], rb, op=ALU.mult
                        )
                    for b in range(UK, NS):
                        nc.scalar.activation(
                            outv[:, b],
                            in0[:, b],
                            AF.Copy,
                            scale=rinv[:, b : b + 1],
                        )
                    us[tt] = u

                # --- stage SQRT + RECIP ---
                tt = t - L_SQRT
                if 0 <= tt < TOT:
                    ss = sss.pop(tt)
                    ssap = ss if SS_PE else ss[:]
                    nrm = stats.tile([128, NS], F32, tag="nrm")
                    nc.scalar.sqrt(nrm[:], ssap)
                    rinv = stats.tile([128, NS], F32, tag="rinv")
                    nc.vector.reciprocal(rinv[:], nrm[:])
                    nrms[tt], rinvs[tt] = nrm, rinv

                # --- stage SQ + ssred ---
                tt = t - L_SQ
                if 0 <= tt < TOT:
                    if SS_PE:
                        sqc = sqcs.pop(tt)
                        r = tt % SSR
                        for b in range(NS):
                            nc.tensor.matmul(
                                ss_all[:, r, b : b + 1],
                                sqc[:, b, :],
                                ones_c[:],
                                start=True,
                                stop=True,
                            )

                        sss[tt] = ss_all[:, r, :]
                    else:
                        pT = pTs[tt]
                        sq = sqpool.tile(
                            [128, NS, 128], BF16 if SQW_BF16 else F32, tag="sq"
                        )
                        nc.scalar.activation(sq[:], pT[:], AF.Square)
                        ss = stats.tile([128, NS], F32, tag="ss")
                        nc.vector.tensor_reduce(
                            ss[:], sq[:], axis=AX.X, op=ALU.add
                        )
                        sss[tt] = ss

                # --- stage TRANSPOSE (lag L_T) ---
                tt = t - L_T
                if 0 <= tt < TOT:
                    g, k = divmod(tt, NK)
                    pg = ptiles[g]
                    pT = psT.tile([128, NS, 128], PDT, tag="pT")
                    for b in range(NS):
                        nc.tensor.transpose(
                            pT[:, b, :],
                            pg[:, b, k * 128 : (k + 1) * 128],
                            ident[:],
                        )
                    pTs[tt] = pT

                # --- stage 0: outt alloc + channel-layout squares ---
                tt = t
                if tt < TOT:
                    g, k = divmod(tt, NK)
                    if k == 0:
                        outts[g] = outpool.tile(
                            [128, NS, NK, 2, 64],
                            BF16 if OUT_BF16 else F32,
                            tag="ot",
                            name=f"ot_{g}",
                        )
                    if SS_PE:
                        pg = ptiles[g]
                        if k % 4 == 0:
                            pgc = pg[:, :, k * 128 : (k + 4) * 128]
                            sqc4 = sqcpool.tile(
                                [128, NS, 512], BF16, tag="sqc", name=f"sqc_{tt}"
                            )
                            nc.vector.tensor_tensor(sqc4[:], pgc, pgc, op=ALU.mult)
                            for kk2 in range(4):
                                sqcs[tt + kk2] = sqc4[
                                    :, :, kk2 * 128 : (kk2 + 1) * 128
                                ]

    nc.compile()
    return nc


def make_in_maps(shards: np.ndarray) -> list[dict]:
    import ml_dtypes

    ident, wbox = _consts()
    identc = ident.astype(ml_dtypes.bfloat16) if IN_BF16 else ident
    wboxb = wbox.astype(ml_dtypes.bfloat16)
    return [
        {"p": shards[i], "ident": identc, "wbox": wboxb} for i in range(8)
    ]


def kernel(p_vector: np.ndarray) -> np.ndarray:
    p = np.ascontiguousarray(p_vector, dtype=np.float32)
    assert p.shape == (256, 128, 32, 32)
    shards = p.reshape(8, B_PER_CORE, C, S)
    nc = build_nc()
    in_maps = make_in_maps(shards)
    res = run_bass_kernel_spmd(nc, in_maps, core_ids=list(range(8)))
    outs = [
        np.asarray(r["out"], dtype=np.float32).reshape(B_PER_CORE, 2048, 64)
        for r in res.results
    ]
    return np.concatenate(outs, axis=0)


if __name__ == "__main__":
    x = np.random.randn(256, 128, 32, 32).astype(np.float32)
    y = kernel(x)
    print(y.shape, y.dtype)


# revision 31
# speedup vs baseline: 1.0011x; 1.0011x over previous
"""Trainium2 Bass kernel for nn_BCIM_45861660787130 (pooling / box-filter sim).

Math per sample (C=128 channels, 32x32 spatial = S=1024 pixels):
  unit = p / ||p||_C
  wmean = 3x3 zero-padded box mean of unit (per channel)
  sim = <unit, wmean>_C          # per pixel
  out = p * sim, then channel deinterleave c=(f*2+e) -> [e*S + s, f]

Design (per core, data-parallel over batch; 8 samples per group, NS=8):
  - SWDGE cast-DMA loads each group as bf16 [c=128, (b, s)] (two s-half
    DMAs so the pipeline ramps at 1 MB).
  - Flat software pipeline over all 32 chunk-iterations (4 groups x 8
    s-chunks) with explicit stage lags (transpose @t, square/ss @t-1,
    sqrt/rinv @t-2, normalize @t-3, box/dot @t-5, scale-out @t-6) so
    every cross-engine dependency is >=1 iteration old and the in-order
    engine queues never head-of-line block.
  - PE: bf16 transposes (1 cyc/row) -> pT [s,c] PSUM; box filter as
    block-tridiagonal bf16 matmuls box_k = Bd^T u_k + Bp^T u_{k-1} +
    Bn^T u_{k+1} (PSUM accumulate, N=512 halves per bank).
  - ACT: Square [128,1024] -> sq; sqrt -> nrm; 1 normalize copy-scale;
    7 of 8 scale-out copies.  DVE: segmented reduces (ss, z), recip,
    u-normalize as one stride-0-broadcast TT (write AP deinterleaves
    c=(f*2+e) -> (e*64+f) so downstream is contiguous), wscr = u*box.
    Pool: 1 of 8 scale-out slots (Pool ops cost ~2.9us fixed; only one
    slot is profitable).
  - NOTE hard-won HW constraints: tensor_tensor_reduce crashes the
    device; GPSIMD cannot touch PSUM; one matmul's PSUM out must fit a
    2 KB bank.
  - Output staged bf16 (host upcasts); per (sample, e, k-half) HWDGE
    DMAs so the tail drains early.  rel err ~3.6e-3 (bf16 input quant).
"""

import os
import sys

sys.path.insert(0, "/opt/trn_rl_repo")

import numpy as np

from concourse import bacc, bass, mybir, tile
from concourse.bass import broadcast_tensor_aps
from concourse.bass_utils import run_bass_kernel_spmd

F32 = mybir.dt.float32
BF16 = mybir.dt.bfloat16
AF = mybir.ActivationFunctionType
ALU = mybir.AluOpType
AX = mybir.AxisListType

B_PER_CORE = 32  # samples per core
NS = int(os.environ.get("NS", 8))  # samples per group
NG = B_PER_CORE // NS
NG_RUN = int(os.environ.get("NG_RUN", NG))
NK = 8  # s-chunks per sample (1024 / 128)
C = 128
S = 1024

# engine assignment knobs (A=ACT, D=DVE, P=Pool) per sample index
OUT_ASSIGN = (os.environ.get("OUT_ASSIGN", "A" * 16) * 4)[:16]
SS_ENG = os.environ.get("SS_ENG", "D")  # segmented sum-sq reduce: D or P
TTR_ASSIGN = (os.environ.get("TTR_ASSIGN", "D" * 16) * 4)[:16]
IN_BF16 = os.environ.get("IN_BF16", "1") == "1"  # SWDGE cast input + bf16 transposes
U_MODE = os.environ.get("U_MODE", "bcast")  # bcast: one TT w/ stride-0; ts: per-sample
DEINT = os.environ.get("DEINT", "1") == "1"  # deinterleave at u write vs at out read
Z_MODE = os.environ.get("Z_MODE", "split")  # ttr: per-sample TTR (HW-crashes); split: TT+reduce
U_BF16 = os.environ.get("U_BF16", "1") == "1"  # u/wbox dtype bf16 vs f32r
W_ENG = os.environ.get("W_ENG", "D")  # wscr big TT: D=DVE, P=Pool
OUT_BF16 = os.environ.get("OUT_BF16", "1") == "1"  # bf16 DRAM output, host upcast
UK = int(os.environ.get("UK", min(NS, 5)))  # u: first UK samples DVE-bcast, rest ACT
SQW_BF16 = os.environ.get("SQW_BF16", "0") == "1"  # bf16 sq/wscr reduce inputs
SS_PE = os.environ.get("SS_PE", "1") == "1"  # sum-of-squares via PE ones-matmul
OUT_POOL2 = int(os.environ.get("OUT_POOL2", "3"))  # first N out slots as one Pool bcast TT
U_ASSIGN = (os.environ.get("U_ASSIGN", "D" * 16) * 4)[:16]


def _consts():
    t32 = (np.abs(np.subtract.outer(np.arange(32), np.arange(32))) <= 1).astype(
        np.float32
    )
    a4 = (np.abs(np.subtract.outer(np.arange(4), np.arange(4))) <= 1).astype(
        np.float32
    )
    e30 = np.zeros((4, 4), np.float32)
    e30[3, 0] = 1.0
    e03 = np.zeros((4, 4), np.float32)
    e03[0, 3] = 1.0
    bd = np.kron(a4, t32) / 9.0
    bp = np.kron(e30, t32) / 9.0  # from chunk k-1
    bn = np.kron(e03, t32) / 9.0  # from chunk k+1
    ident = np.eye(128, dtype=np.float32)
    wbox = np.stack([bd, bp, bn]).astype(np.float32)
    return ident, wbox


def _bcast(src_ap, like_ap):
    """Broadcast src_ap (fewer/size-1 dims) against like_ap via stride-0."""
    a, b = broadcast_tensor_aps(like_ap, src_ap)
    return b


def build_nc():
    nc = bacc.Bacc()
    p_d = nc.declare_dram_parameter("p", [B_PER_CORE, C, S], F32, isOutput=False)
    ODT = BF16 if OUT_BF16 else F32
    out_d = nc.declare_dram_parameter(
        "out", [B_PER_CORE, 2, NK, 128, 64], ODT, isOutput=True
    )
    IDT0 = BF16 if IN_BF16 else F32
    ident_d = nc.declare_dram_parameter("ident", [128, 128], IDT0, isOutput=False)
    MMDT = BF16 if U_BF16 else mybir.dt.float32r
    wbox_d = nc.declare_dram_parameter("wbox", [3, 128, 128], MMDT, isOutput=False)

    with tile.TileContext(nc) as tc:
        with (
            tc.tile_pool(name="consts", bufs=1) as cpool,
            tc.tile_pool(name="pin", bufs=4) as pin,
            tc.tile_pool(name="upool", bufs=10) as upool,
            tc.tile_pool(name="sq", bufs=3) as sqpool,
            tc.tile_pool(name="wscr", bufs=3) as wpool,
            tc.tile_pool(name="outp", bufs=3) as outpool,
            tc.tile_pool(name="stats", bufs=6 * NK) as stats,
            tc.tile_pool(name="sqc", bufs=3) as sqcpool,
            tc.tile_pool(name="psT", bufs=3 if SS_PE else 4, space="PSUM") as psT,
            tc.tile_pool(name="psB", bufs=2, space="PSUM") as psB,
            tc.tile_pool(name="psS", bufs=1, space="PSUM") as psS,
        ):
            IDT = BF16 if IN_BF16 else F32
            ident = cpool.tile([128, 128], IDT, tag="ident")
            wbox = cpool.tile([128, 3, 128], MMDT, tag="wbox")
            nc.sync.dma_start(ident[:], ident_d[:])
            nc.sync.dma_start(wbox[:], wbox_d[:].transpose([1, 0, 2]))
            bd, bp, bn = wbox[:, 0, :], wbox[:, 1, :], wbox[:, 2, :]
            ones_c = cpool.tile([128, 1], BF16, tag="ones")
            nc.vector.memset(ones_c[:], 1.0)

            # startup observers: make PE's vector clock see both const-DMA
            # queue sems so steady-state matmuls never wait on them.
            scr1 = psB.tile([128, 1], F32, tag="box")
            nc.tensor.matmul(scr1[:], ident[:], ident[:, 0:1], start=True, stop=True)
            scr2 = psB.tile([128, 1], F32, tag="box")
            nc.tensor.matmul(
                scr2[:], wbox[:, 0, :], wbox[:, 0, 0:1], start=True, stop=True
            )

            PDT = BF16 if IN_BF16 else F32
            ptiles = []
            for g in range(NG_RUN):
                pg = pin.tile([C, NS, S], PDT, tag="pg", name=f"pg_{g}")
                src_ap = p_d[g * NS : (g + 1) * NS].transpose([1, 0, 2])
                H = S // 2
                for h in range(2):
                    sl = slice(h * H, (h + 1) * H)
                    if IN_BF16:
                        # SWDGE cast f32 -> bf16 during HBM->SBUF
                        nc.gpsimd.dma_start(pg[:, :, sl], src_ap[:, :, sl])
                    else:
                        nc.sync.dma_start(pg[:, :, sl], src_ap[:, :, sl])
                ptiles.append(pg)

            # flat software pipeline over all chunks t=(g,k); stage lags
            # keep every cross-engine dep >=1 iteration old so per-engine
            # in-order queues never head-of-line block.
            TOT = NG_RUN * NK
            L_SQ, L_SQRT, L_U, L_BOX, L_OUT = 1, 2, 3, 5, 7
            L_BW = 6
            boxes = {}
            pTs, sss, nrms, rinvs, us, zs, fss = {}, {}, {}, {}, {}, {}, {}
            sqcs = {}
            outts = {}
            SSR = 4
            ss_all = (
                psS.tile([128, SSR, NS], F32, tag="ssall", name="ss_all")
                if SS_PE
                else None
            )
            L_T = 2 if SS_PE else 0

            for t in range(TOT + L_OUT + 1):
                # --- stage OUT (oldest first for queue friendliness) ---
                tt = t - L_OUT
                if 0 <= tt < TOT:
                    g, k = divmod(tt, NK)
                    fs = fss.pop(tt)
                    u = us[tt]
                    outt = outts[g]
                    if OUT_POOL2 > 0:
                        nb = OUT_POOL2
                        ob = outt[:, 0:nb, k, :, :]
                        ub = u[:, 0:nb, :].rearrange("p b (e f) -> p b e f", e=2)
                        fb = (
                            fs[:, 0:nb]
                            .unsqueeze(2)
                            .unsqueeze(3)
                            .broadcast_to([128, nb, 2, 64])
                        )
                        nc.gpsimd.tensor_tensor(ob, ub, fb, op=ALU.mult)
                    for b in range(OUT_POOL2, NS):
                        ov = outt[:, b, k, :, :].rearrange("p e f -> p (e f)")
                        uv = u[:, b, :]
                        if not U_BF16:
                            uv = uv.bitcast(F32)
                        eng = OUT_ASSIGN[b]
                        if eng == "A":
                            nc.scalar.activation(
                                ov, uv, AF.Copy, scale=fs[:, b : b + 1]
                            )
                        elif eng == "P":
                            nc.gpsimd.tensor_scalar_mul(ov, uv, fs[:, b : b + 1])
                        else:
                            nc.vector.tensor_scalar_mul(ov, uv, fs[:, b : b + 1])
                    last_g = g == NG_RUN - 1
                    flush = []
                    if last_g and k == NK // 2 - 1:
                        flush = [slice(0, NK // 2)]
                    elif last_g and k == NK - 1:
                        flush = [slice(NK // 2, NK)]
                    elif not last_g and k == NK - 1:
                        flush = [slice(0, NK)]
                    for kl in flush:
                        for b in range(NS):
                            for e in range(2):
                                dst = out_d[g * NS + b, e, kl].transpose([1, 0, 2])
                                nc.sync.dma_start(dst, outt[:, b, kl, e, :])
                    if k == NK - 1:
                        for j in range(max(0, tt - NK), tt + 1):
                            us.pop(j, None)

                # --- stage WSCR + zred + fs (box is 1 iter old) ---
                tt = t - L_BW
                if 0 <= tt < TOT:
                    box = boxes.pop(tt)
                    z = stats.tile([128, NS], F32, tag="z")
                    wscr = wpool.tile(
                        [128, NS, 128], BF16 if SQW_BF16 else F32, tag="w"
                    )
                    uk = us[tt][:]
                    if not U_BF16:
                        uk = uk.bitcast(F32)
                    nc.vector.tensor_tensor(wscr[:], uk, box[:], op=ALU.mult)
                    nc.vector.tensor_reduce(z[:], wscr[:], axis=AX.X, op=ALU.add)
                    fs = stats.tile([128, NS], F32, tag="fs")
                    nc.vector.tensor_mul(fs[:], z[:], nrms[tt][:])
                    fss[tt] = fs

                # --- stage BOX matmuls ---
                tt = t - L_BOX
                if 0 <= tt < TOT:
                    g, k = divmod(tt, NK)
                    box = psB.tile([128, NS, 128], F32, tag="box")
                    mms = [(bd, tt)]
                    if k > 0:
                        mms.append((bp, tt - 1))
                    if k < NK - 1:
                        mms.append((bn, tt + 1))
                    for h in range(0, NS, 4):
                        sl = slice(h, min(h + 4, NS))
                        for i, (w, j) in enumerate(mms):
                            nc.tensor.matmul(
                                box[:, sl, :],
                                w,
                                us[j][:, sl, :],
                                start=(i == 0),
                                stop=(i == len(mms) - 1),
                            )
                    boxes[tt] = box

                # --- stage U (normalize) ---
                tt = t - L_U
                if 0 <= tt < TOT:
                    g, k = divmod(tt, NK)
                    pT = pTs.pop(tt)
                    rinv = rinvs.pop(tt)
                    u = upool.tile(
                        [128, NS, 128], MMDT, tag="u", name=f"u_{tt}"
                    )
                    in0 = pT[:].rearrange("p b (f e) -> p b e f", e=2)
                    outv = u[:].rearrange("p b (e f) -> p b e f", e=2)
                    if UK > 0:
                        rb = (
                            rinv[:, 0:UK]
                            .unsqueeze(2)
                            .unsqueeze(3)
                            .broadcast_to([128, UK, 2, 64])
                        )
                        nc.vector.tensor_tensor(
                            outv[:, 0:UK], in0[:, 0:UK], rb, op=ALU.mult
                        )
                    for b in range(UK, NS):
                        nc.scalar.activation(
                            outv[:, b],
                            in0[:, b],
                            AF.Copy,
                            scale=rinv[:, b : b + 1],
                        )
                    us[tt] = u

                # --- stage SQRT + RECIP ---
                tt = t - L_SQRT
                if 0 <= tt < TOT:
                    ss = sss.pop(tt)
                    ssap = ss if SS_PE else ss[:]
                    nrm = stats.tile([128, NS], F32, tag="nrm")
                    nc.scalar.sqrt(nrm[:], ssap)
                    rinv = stats.tile([128, NS], F32, tag="rinv")
                    nc.vector.reciprocal(rinv[:], nrm[:])
                    nrms[tt], rinvs[tt] = nrm, rinv

                # --- stage SQ + ssred ---
                tt = t - L_SQ
                if 0 <= tt < TOT:
                    if SS_PE:
                        sqc = sqcs.pop(tt)
                        r = tt % SSR
                        for b in range(NS):
                            nc.tensor.matmul(
                                ss_all[:, r, b : b + 1],
                                sqc[:, b, :],
                                ones_c[:],
                                start=True,
                                stop=True,
                            )

                        sss[tt] = ss_all[:, r, :]
                    else:
                        pT = pTs[tt]
                        sq = sqpool.tile(
                            [128, NS, 128], BF16 if SQW_BF16 else F32, tag="sq"
                        )
                        nc.scalar.activation(sq[:], pT[:], AF.Square)
                        ss = stats.tile([128, NS], F32, tag="ss")
                        nc.vector.tensor_reduce(
                            ss[:], sq[:], axis=AX.X, op=ALU.add
                        )
                        sss[tt] = ss

                # --- stage TRANSPOSE (lag L_T) ---
                tt = t - L_T
                if 0 <= tt < TOT:
                    g, k = divmod(tt, NK)
                    pg = ptiles[g]
                    pT = psT.tile([128, NS, 128], PDT, tag="pT")
                    for b in range(NS):
                        nc.tensor.transpose(
                            pT[:, b, :],
                            pg[:, b, k * 128 : (k + 1) * 128],
                            ident[:],
                        )
                    pTs[tt] = pT

                # --- stage 0: outt alloc + channel-layout squares ---
                tt = t
                if tt < TOT:
                    g, k = divmod(tt, NK)
                    if k == 0:
                        outts[g] = outpool.tile(
                            [128, NS, NK, 2, 64],
                            BF16 if OUT_BF16 else F32,
                            tag="ot",
                            name=f"ot_{g}",
                        )
                    if SS_PE:
                        pg = ptiles[g]
                        if k % 4 == 0:
                            pgc = pg[:, :, k * 128 : (k + 4) * 128]
                            sqc4 = sqcpool.tile(
                                [128, NS, 512], BF16, tag="sqc", name=f"sqc_{tt}"
                            )
                            nc.vector.tensor_tensor(sqc4[:], pgc, pgc, op=ALU.mult)
                            for kk2 in range(4):
                                sqcs[tt + kk2] = sqc4[
                                    :, :, kk2 * 128 : (kk2 + 1) * 128
                                ]

    nc.compile()
    return nc


def make_in_maps(shards: np.ndarray) -> list[dict]:
    import ml_dtypes

    ident, wbox = _consts()
    identc = ident.astype(ml_dtypes.bfloat16) if IN_BF16 else ident
    wboxb = wbox.astype(ml_dtypes.bfloat16)
    return [
        {"p": shards[i], "ident": identc, "wbox": wboxb} for i in range(8)
    ]


def kernel(p_vector: np.ndarray) -> np.ndarray:
    p = np.ascontiguousarray(p_vector, dtype=np.float32)
    assert p.shape == (256, 128, 32, 32)
    shards = p.reshape(8, B_PER_CORE, C, S)
    nc = build_nc()
    in_maps = make_in_maps(shards)
    res = run_bass_kernel_spmd(nc, in_maps, core_ids=list(range(8)))
    outs = [
        np.asarray(r["out"], dtype=np.float32).reshape(B_PER_CORE, 2048, 64)
        for r in res.results
    ]
    return np.concatenate(outs, axis=0)


if __name__ == "__main__":
    x = np.random.randn(256, 128, 32, 32).astype(np.float32)
    y = kernel(x)
    print(y.shape, y.dtype)
